# revision 1
# baseline (speedup 1.0000x reference)
"""Trainium2 Bass kernel for nn_Concat_84653805404632.

Reference computation: x is [70, 128, 512] f32; rows 0..19 are supports
(ns_all = n_class*n_support = 20), rows 20..69 are queries (nq_all = 50).
Output [1000, 128, 1024] where out[q*20+s] = concat(sup[s], qry[q], axis=-1).

Pure data movement (memory regime). Sharding: the (query, support) pair grid
[50 x 20] is split as (2 query-halves) x (4 support-fifths) -> 8 cores, each
producing exactly 125 output rows (64 MB) with an identical SPMD access
pattern. Each core stages its 25 query tiles + 5 support tiles in SBUF, then
streams the output with 50 large (1.31 MB) strided DMA writes; the query
halves use a stride-0 (broadcast) source AP so each query tile is written 5x
without SBUF replication.
"""

import os
import sys

import numpy as np

for _p in ("/opt/trn_rl_repo", "/root/.axon_site/_ro/trn_rl_repo"):
    if os.path.isdir(_p) and _p not in sys.path:
        sys.path.insert(0, _p)

import concourse.bass as bass
import concourse.mybir as mybir
from concourse.bass_utils import run_bass_kernel_spmd

NS_ALL = 20  # n_class * n_support
NQ_ALL = 50  # n_class * n_query
D = 128
F = 512
QH = 25  # queries per core  (NQ_ALL / 2)
SF = 5  # supports per core (NS_ALL / 4)
N_CORES = 8

_NC_CACHE = None


def _build_nc():
    nc = bass.Bass()
    sup = nc.declare_dram_parameter("sup", [SF, D, F], mybir.dt.float32, isOutput=False)
    qry = nc.declare_dram_parameter("qry", [QH, D, F], mybir.dt.float32, isOutput=False)
    out = nc.declare_dram_parameter(
        "out", [QH * SF, D, 2 * F], mybir.dt.float32, isOutput=True
    )

    with (
        nc.sbuf_tensor([D, SF * F], mybir.dt.float32) as sup_t,
        nc.sbuf_tensor([D, QH * F], mybir.dt.float32) as qry_t,
        nc.semaphore("sup_sem") as sup_sem,
        nc.semaphore("qry_sem") as qry_sem,
        nc.semaphore("out_sem") as out_sem,
        nc.Block() as block,
    ):

        @block.sync
        def _(sync):
            sync.dma_start(sup_t[:], sup[:].transpose([1, 0, 2])).then_inc(sup_sem, 16)
            sync.dma_start(qry_t[:], qry[:].transpose([1, 0, 2])).then_inc(qry_sem, 16)
            # Support halves first: they only need the small sup load, so the
            # 25 writes (32.8 MB) overlap the in-flight 6.5 MB query load.
            sync.wait_ge(sup_sem, 16)
            for q in range(QH):
                dst = out[SF * q : SF * (q + 1), :, 0:F].transpose([1, 0, 2])
                sync.dma_start(dst, sup_t[:]).then_inc(out_sem, 16)
            sync.wait_ge(qry_sem, 16)
            for q in range(QH):
                dst = out[SF * q : SF * (q + 1), :, F : 2 * F].transpose([1, 0, 2])
                src = (
                    qry_t[:, F * q : F * (q + 1)]
                    .unsqueeze(1)
                    .broadcast_to([D, SF, F])
                )
                sync.dma_start(dst, src).then_inc(out_sem, 16)
            sync.wait_ge(out_sem, 2 * QH * 16)
    return nc


def _get_nc():
    global _NC_CACHE
    if _NC_CACHE is None:
        _NC_CACHE = _build_nc()
    return _NC_CACHE


def kernel(**inputs) -> np.ndarray:
    x = np.ascontiguousarray(np.asarray(inputs["x"], dtype=np.float32))
    assert x.shape == (NS_ALL + NQ_ALL, D, F), x.shape

    sup_all = x[:NS_ALL]
    qry_all = x[NS_ALL:]

    in_maps = []
    for k in range(N_CORES):
        h, f = divmod(k, 4)
        in_maps.append(
            {
                "sup": np.ascontiguousarray(sup_all[SF * f : SF * (f + 1)]),
                "qry": np.ascontiguousarray(qry_all[QH * h : QH * (h + 1)]),
            }
        )

    nc = _get_nc()
    res = run_bass_kernel_spmd(nc, in_maps, core_ids=list(range(N_CORES)))

    full = np.empty((NQ_ALL, NS_ALL, D, 2 * F), dtype=np.float32)
    for k in range(N_CORES):
        h, f = divmod(k, 4)
        out_k = np.asarray(res.results[k]["out"]).reshape(QH, SF, D, 2 * F)
        full[QH * h : QH * (h + 1), SF * f : SF * (f + 1)] = out_k
    return full.reshape(NQ_ALL * NS_ALL, D, 2 * F)


# revision 2
# speedup vs baseline: 1.0070x; 1.0070x over previous
"""Trainium2 Bass kernel for nn_Concat_84653805404632.

Reference computation: x is [70, 128, 512] f32; rows 0..19 are supports
(ns_all = n_class*n_support = 20), rows 20..69 are queries (nq_all = 50).
Output [1000, 128, 1024] where out[q*20+s] = concat(sup[s], qry[q], axis=-1).

Pure data movement (memory regime). Sharding: the (query, support) pair grid
[50 x 20] is split as (2 query-halves) x (4 support-fifths) -> 8 cores, each
producing exactly 125 output rows (64 MB) with an identical SPMD access
pattern. Each core stages its 25 query tiles + 5 support tiles in SBUF, then
streams the output with 50 large (1.31 MB) strided DMA writes; the query
halves use a stride-0 (broadcast) source AP so each query tile is written 5x
without SBUF replication.
"""

import os
import sys

import numpy as np

for _p in ("/opt/trn_rl_repo", "/root/.axon_site/_ro/trn_rl_repo"):
    if os.path.isdir(_p) and _p not in sys.path:
        sys.path.insert(0, _p)

import concourse.bass as bass
import concourse.mybir as mybir
from concourse.bass_utils import run_bass_kernel_spmd

NS_ALL = 20  # n_class * n_support
NQ_ALL = 50  # n_class * n_query
D = 128
F = 512
QH = 25  # queries per core  (NQ_ALL / 2)
SF = 5  # supports per core (NS_ALL / 4)
N_CORES = 8

_NC_CACHE = None


def _build_nc():
    nc = bass.Bass()
    sup = nc.declare_dram_parameter("sup", [SF, D, F], mybir.dt.float32, isOutput=False)
    qry = nc.declare_dram_parameter("qry", [QH, D, F], mybir.dt.float32, isOutput=False)
    out = nc.declare_dram_parameter(
        "out", [QH * SF, D, 2 * F], mybir.dt.float32, isOutput=True
    )

    with (
        nc.sbuf_tensor([D, SF * F], mybir.dt.float32) as sup_t,
        nc.sbuf_tensor([D, QH * F], mybir.dt.float32) as qry_t,
        nc.semaphore("sup_sem") as sup_sem,
        nc.semaphore("qry_sem") as qry_sem,
        nc.semaphore("sup_out_sem") as sup_out_sem,
        nc.semaphore("qry_out_sem") as qry_out_sem,
        nc.Block() as block,
    ):
        # Two HWDGE rings (sync + scalar) feed the same 16 SDMA engines; the
        # engines round-robin between rings at packet granularity, so the two
        # write streams interleave and the big query load doesn't FIFO-block
        # the support-half writes.

        @block.sync
        def _(sync):
            sync.dma_start(sup_t[:], sup[:].transpose([1, 0, 2])).then_inc(sup_sem, 16)
            sync.wait_ge(sup_sem, 16)
            for q in range(QH):
                dst = out[SF * q : SF * (q + 1), :, 0:F].transpose([1, 0, 2])
                sync.dma_start(dst, sup_t[:]).then_inc(sup_out_sem, 16)
            sync.wait_ge(sup_out_sem, QH * 16)

        @block.scalar
        def _(scalar):
            scalar.dma_start(qry_t[:], qry[:].transpose([1, 0, 2])).then_inc(
                qry_sem, 16
            )
            scalar.wait_ge(qry_sem, 16)
            for q in range(QH):
                dst = out[SF * q : SF * (q + 1), :, F : 2 * F].transpose([1, 0, 2])
                src = (
                    qry_t[:, F * q : F * (q + 1)]
                    .unsqueeze(1)
                    .broadcast_to([D, SF, F])
                )
                scalar.dma_start(dst, src).then_inc(qry_out_sem, 16)
            scalar.wait_ge(qry_out_sem, QH * 16)
    return nc


def _get_nc():
    global _NC_CACHE
    if _NC_CACHE is None:
        _NC_CACHE = _build_nc()
    return _NC_CACHE


def kernel(**inputs) -> np.ndarray:
    x = np.ascontiguousarray(np.asarray(inputs["x"], dtype=np.float32))
    assert x.shape == (NS_ALL + NQ_ALL, D, F), x.shape

    sup_all = x[:NS_ALL]
    qry_all = x[NS_ALL:]

    in_maps = []
    for k in range(N_CORES):
        h, f = divmod(k, 4)
        in_maps.append(
            {
                "sup": np.ascontiguousarray(sup_all[SF * f : SF * (f + 1)]),
                "qry": np.ascontiguousarray(qry_all[QH * h : QH * (h + 1)]),
            }
        )

    nc = _get_nc()
    res = run_bass_kernel_spmd(nc, in_maps, core_ids=list(range(N_CORES)))

    full = np.empty((NQ_ALL, NS_ALL, D, 2 * F), dtype=np.float32)
    for k in range(N_CORES):
        h, f = divmod(k, 4)
        out_k = np.asarray(res.results[k]["out"]).reshape(QH, SF, D, 2 * F)
        full[QH * h : QH * (h + 1), SF * f : SF * (f + 1)] = out_k
    return full.reshape(NQ_ALL * NS_ALL, D, 2 * F)


# revision 3
# speedup vs baseline: 1.0232x; 1.0161x over previous
"""Trainium2 Bass kernel for nn_Concat_84653805404632.

Reference computation: x is [70, 128, 512] f32; rows 0..19 are supports
(ns_all = n_class*n_support = 20), rows 20..69 are queries (nq_all = 50).
Output [1000, 128, 1024] where out[q*20+s] = concat(sup[s], qry[q], axis=-1).

Pure data movement (memory regime). Sharding: the (query, support) pair grid
[50 x 20] is split as (2 query-halves) x (4 support-fifths) -> 8 cores, each
producing exactly 125 output rows (64 MB) with an identical SPMD access
pattern.

Per core: the 5 support tiles are DMA-loaded directly into the sup columns of
two interleaved "image" buffers in SBUF; the VectorEngine broadcasts each
query tile into the qry columns (SBUF engine ports are separate from the DMA
AXI ports, so this is free); each query then leaves as ONE 2.62 MB write DMA
whose descriptors are full 4 KB rows. 4 KB descriptors matter: SDMA engine 15
has ~15 ns/packet extra fixed cost and the in-order descriptor generator
stalls on its ring, so the whole core runs at engine-15's packet rate —
bigger packets raise that ceiling from ~325 GB/s to ~390 GB/s.
"""

import os
import sys

import numpy as np

for _p in ("/opt/trn_rl_repo", "/root/.axon_site/_ro/trn_rl_repo"):
    if os.path.isdir(_p) and _p not in sys.path:
        sys.path.insert(0, _p)

import concourse.bass as bass
import concourse.mybir as mybir
from concourse.bass_utils import run_bass_kernel_spmd

NS_ALL = 20  # n_class * n_support
NQ_ALL = 50  # n_class * n_query
D = 128
F = 512
QH = 25  # queries per core  (NQ_ALL / 2)
SF = 5  # supports per core (NS_ALL / 4)
QCH = 5  # query tiles per load chunk
N_CORES = 8

_NC_CACHE = None


def _build_nc():
    nc = bass.Bass()
    sup = nc.declare_dram_parameter("sup", [SF, D, F], mybir.dt.float32, isOutput=False)
    qry = nc.declare_dram_parameter("qry", [QH, D, F], mybir.dt.float32, isOutput=False)
    out = nc.declare_dram_parameter(
        "out", [QH * SF, D, 2 * F], mybir.dt.float32, isOutput=True
    )

    with (
        nc.sbuf_tensor([D, QH * F], mybir.dt.float32) as qry_t,
        nc.sbuf_tensor([D, SF * 2 * F], mybir.dt.float32) as img0,
        nc.sbuf_tensor([D, SF * 2 * F], mybir.dt.float32) as img1,
        nc.semaphore("img_sup_sem0") as img_sup_sem0,
        nc.semaphore("img_sup_sem1") as img_sup_sem1,
        nc.semaphore("qry_sem0") as qry_sem0,
        nc.semaphore("qry_sem1") as qry_sem1,
        nc.semaphore("qry_sem2") as qry_sem2,
        nc.semaphore("qry_sem3") as qry_sem3,
        nc.semaphore("qry_sem4") as qry_sem4,
        nc.semaphore("dve_sem") as dve_sem,
        nc.semaphore("out_sem0") as out_sem0,
        nc.semaphore("out_sem1") as out_sem1,
        nc.Block() as block,
    ):
        imgs = [img0, img1]
        sup_sems = [img_sup_sem0, img_sup_sem1]
        qry_sems = [qry_sem0, qry_sem1, qry_sem2, qry_sem3, qry_sem4]
        out_sems = [out_sem0, out_sem1]

        def img_view(b):
            # [p, s, f2] view of the 5-row interleaved image (f2 = 1024)
            return imgs[b][:].rearrange("p (s f2) -> p s f2", f2=2 * F)

        @block.sync
        def _(sync):
            # Support tiles straight into the sup columns of both images,
            # then the query tiles in chunks (per-chunk semaphores: DMA
            # completions are unordered).
            for b in range(2):
                sync.dma_start(
                    img_view(b)[:, :, 0:F], sup[:].transpose([1, 0, 2])
                ).then_inc(sup_sems[b], 16)
            for c in range(QH // QCH):
                sync.dma_start(
                    qry_t[:, QCH * F * c : QCH * F * (c + 1)],
                    qry[QCH * c : QCH * (c + 1)].transpose([1, 0, 2]),
                ).then_inc(qry_sems[c], 16)

        @block.vector
        def _(vector):
            for q in range(QH):
                vector.wait_ge(qry_sems[q // QCH], 16)
                if q >= 2:
                    # img[q%2] free once all issued writes on it are done.
                    vector.wait_ge(out_sems[q % 2], 16 * (q // 2))
                dst = img_view(q % 2)[:, :, F : 2 * F]
                src = (
                    qry_t[:, F * q : F * (q + 1)]
                    .unsqueeze(1)
                    .broadcast_to([D, SF, F])
                )
                vector.tensor_copy(dst, src).then_inc(dve_sem, 1)

        @block.scalar
        def _(scalar):
            for q in range(QH):
                if q == 0:
                    scalar.wait_ge(img_sup_sem0, 16)
                elif q == 1:
                    scalar.wait_ge(img_sup_sem1, 16)
                scalar.wait_ge(dve_sem, q + 1)
                dst = out[SF * q : SF * (q + 1), :, :].transpose([1, 0, 2])
                scalar.dma_start(dst, imgs[q % 2][:]).then_inc(out_sems[q % 2], 16)
            scalar.wait_ge(out_sem0, 16 * ((QH + 1) // 2))
            scalar.wait_ge(out_sem1, 16 * (QH // 2))

    return nc


def _get_nc():
    global _NC_CACHE
    if _NC_CACHE is None:
        _NC_CACHE = _build_nc()
    return _NC_CACHE


def kernel(**inputs) -> np.ndarray:
    x = np.ascontiguousarray(np.asarray(inputs["x"], dtype=np.float32))
    assert x.shape == (NS_ALL + NQ_ALL, D, F), x.shape

    sup_all = x[:NS_ALL]
    qry_all = x[NS_ALL:]

    in_maps = []
    for k in range(N_CORES):
        h, f = divmod(k, 4)
        in_maps.append(
            {
                "sup": np.ascontiguousarray(sup_all[SF * f : SF * (f + 1)]),
                "qry": np.ascontiguousarray(qry_all[QH * h : QH * (h + 1)]),
            }
        )

    nc = _get_nc()
    res = run_bass_kernel_spmd(nc, in_maps, core_ids=list(range(N_CORES)))

    full = np.empty((NQ_ALL, NS_ALL, D, 2 * F), dtype=np.float32)
    for k in range(N_CORES):
        h, f = divmod(k, 4)
        out_k = np.asarray(res.results[k]["out"]).reshape(QH, SF, D, 2 * F)
        full[QH * h : QH * (h + 1), SF * f : SF * (f + 1)] = out_k
    return full.reshape(NQ_ALL * NS_ALL, D, 2 * F)
